# revision 14
# baseline (speedup 1.0000x reference)
"""Trainium2 Bass kernel for nn_ChannelMambaBlock.

Math (per pixel, channel vector x of size C=192):
  xn  = LN(x)*nw + nb
  p   = w_in @ xn              ; x1, x2 = p[:C], p[C:]
  u   = ssm_in @ x1            ; xss, z = silu(u[:C]), silu(u[C:])
  xd  = x_proj @ xss (K dirs)  ; dtr, Bv, Cv
  dt  = softplus(dt_w @ dtr + dt_b)
  bc_k = sum_s Bv*Cv ; gain = sum_k bc_k*dt_k + sum_k D_k
  y   = LN2(xss*gain)*ow + ob ; y *= z
  s   = ssm_out @ y ; o = w_out @ (s * silu(x2)) ; out = x + o

Kernel strategy: channel-major [C, pixels] layout, pure data parallel over
8 cores (each owns B*H*W/8 = 8192 pixels, tiles of 512 pixels).
  - LN affine + w_in + ssm_in folded on host into one [576, C] matmul
    producing [xss_pre, z_pre, x2_pre]; per-channel biases ride as
    per-partition ACT bias operands.
  - M-chunks of that matmul coincide with consumer groups (128+64 per
    group) so every elementwise op is partition-aligned (DVE cannot move
    data across partitions).
  - LN stats via PE ones-matmul; rsqrt via exp(-0.5*ln(var+eps)) on ACT.
  - silu via exp(-v) on ACT + reciprocal_approx_fast on DVE (single ACT
    table set: natural_log_exp_and_others -> no table switching).
  - softplus via ACT: Ln(Exp(dtpre + dt_b) + 1).
  - dt block-diag matmul in c-major row order (row = c*4+k); gain k-sum
    on PE with 0/1 selector matmuls; bc broadcast to the c*4+k pattern
    with one selector matmul.
  - Per-pixel scalars (rstd, mu*rstd) broadcast across partitions with
    GPSIMD partition_broadcast.
"""

import numpy as np

C = 192
K = 4
DT = 12
DS = 8
B, H, W = 4, 128, 128
EPS = 1e-5
NCORES = 8
NPIX = B * H * W // NCORES      # 8192 per core
NT = 512                        # pixels per tile
NTILES = NPIX // NT             # 16

# M-chunks of the big fused matmul [u(384); x2(192)]:
#   q0 xss[0:128], q1 xss[128:192], q2 z[0:128], q3 z[128:192],
#   q4 x2[0:128], q5 x2[128:192]
QCH = [(0, 128), (128, 192), (192, 320), (320, 384), (384, 512), (512, 576)]

NPCOL = 24

_CACHE = {}


def _fold_weights(norm_w, norm_b, w_in, ssm_in_w, x_proj_w, dt_w, dt_b,
                  A_logs, Ds, out_norm_w, out_norm_b, ssm_out_w, w_out):
    f8 = np.float64
    nw, nb = norm_w.astype(f8), norm_b.astype(f8)
    w_in = w_in.astype(f8)
    ssm = ssm_in_w.astype(f8)
    W1 = w_in * nw[None, :]
    b1 = w_in @ nb
    W_u = ssm @ W1[:C]                      # (2C, C)
    b_u = ssm @ b1[:C]                      # (2C,)
    W1b = W1[C:]                            # (C, C)
    b_x2 = b1[C:]
    W_big = np.concatenate([W_u, W1b], 0)   # (576, C)
    b_big = np.concatenate([b_u, b_x2], 0)  # (576,)
    # x_proj main rows: [dtr (48, k-major) ; pad (16) ; Bv (32)] = 96
    Xpm = np.concatenate([
        x_proj_w[:, :DT].reshape(K * DT, C),
        np.zeros((16, C)),
        x_proj_w[:, DT:DT + DS].reshape(K * DS, C)], 0).astype(f8)   # (96, C)
    # Cv matmul: rows [pad(64) ; Cv(32)] so Cv lands on partitions 64:96
    Xpc = np.concatenate([
        np.zeros((64, C)),
        x_proj_w[:, DT + DS:].reshape(K * DS, C)], 0).astype(f8)     # (96, C)
    # dt block: out row = c*4 + k, in col = k*12 + r
    Wdt = np.zeros((C * K, K * DT))
    for k in range(K):
        Wdt[np.arange(C) * K + k, k * DT:(k + 1) * DT] = dt_w[k].astype(f8)
    dtb_s = dt_b.astype(f8).T.reshape(C * K)  # row c*4+k
    Dsum = Ds.astype(f8).reshape(K, C).sum(0)  # (C,)
    ow, ob = out_norm_w.astype(f8), out_norm_b.astype(f8)
    Wsp = ssm_out_w.astype(f8) * ow[None, :]
    obp = ob / ow
    Wo = w_out.astype(f8)

    # bc pattern broadcast: bcm[m] = bc[m%4]; lhsT rows at partitions 64:96
    # (matching where Bv*Cv sits).
    combT = np.zeros((96, 128))
    for k in range(K):
        rows = 64 + k * DS + np.arange(DS)
        combT[np.ix_(rows, np.arange(128)[np.arange(128) % 4 == k])] = 1.0
    # gain selectors: chunk j of dt rows [128j,128j+128) covers c in
    # [32j, 32j+32): gain_sel_j[p, 32j + p//4] = 1
    selsA = []
    for j in range(4):
        S = np.zeros((128, 128))
        S[np.arange(128), 32 * j + np.arange(128) // 4] = 1.0
        selsA.append(S)
    selsB = []
    for j in range(2):
        S = np.zeros((128, 64))
        S[np.arange(128), 32 * j + np.arange(128) // 4] = 1.0
        selsB.append(S)

    def f32(a):
        return np.ascontiguousarray(np.asarray(a, np.float32))

    wts = {
        "wbigT": f32(W_big.T),               # (192, 576)
        "xpmT": f32(Xpm.T),                  # (192, 96)
        "xpcT": f32(Xpc.T),                  # (192, 96)
        "wdtT": f32(Wdt.T),                  # (48, 768)
        "combT": f32(combT),                 # (96, 128)
        "selsA": f32(np.concatenate(selsA, 1)),   # (128, 512)
        "selsB": f32(np.concatenate(selsB, 1)),   # (128, 128)
        "wspT": f32(Wsp.T),                  # (192, 192)
        "woT": f32(Wo.T),                    # (192, 192)
        "onesc": f32(np.ones((C, 1))),       # (192, 1) stats lhsT
    }
    cols = []

    def col(v):
        v = np.asarray(v, np.float64).reshape(-1)
        c = np.zeros(128)
        c[:v.size] = v
        cols.append(c)
        return len(cols) - 1

    ci = {}
    for m, (r0, r1) in enumerate(QCH):
        ci[f"negb{m}"] = col(-b_big[r0:r1])
        ci[f"b{m}"] = col(b_big[r0:r1])
    for j in range(6):
        ci[f"dtb{j}"] = col(dtb_s[128 * j:128 * (j + 1)])
    ci["DsA"] = col(Dsum[:128]); ci["DsB"] = col(Dsum[128:])
    ci["obpA"] = col(obp[:128]); ci["obpB"] = col(obp[128:])
    ci["ceps"] = col(np.full(128, C * EPS))
    ci["hlnc"] = col(np.full(128, 0.5 * np.log(C)))
    assert len(cols) == NPCOL, len(cols)
    wts["pcol"] = f32(np.stack(cols, 1))     # (128, NPCOL)
    return wts, ci


def _build_program():
    import concourse.mybir as mybir
    from concourse.bacc import Bacc
    from concourse.tile import TileContext
    from concourse.alu_op_type import AluOpType

    AF = mybir.ActivationFunctionType
    OP = AluOpType
    f32 = mybir.dt.float32
    nc = Bacc()

    xin = nc.dram_tensor("xin", (C, NPIX), f32, kind="ExternalInput")
    yout = nc.dram_tensor("yout", (C, NPIX), f32, kind="ExternalOutput")
    dw = {}
    for name, shape in [("wbigT", (C, 576)), ("xpmT", (C, 96)),
                        ("xpcT", (C, 96)),
                        ("wdtT", (48, 768)), ("combT", (96, 128)),
                        ("selsA", (128, 512)), ("selsB", (128, 128)),
                        ("wspT", (C, 192)), ("woT", (C, 192)),
                        ("onesc", (C, 1)), ("pcol", (128, NPCOL))]:
        dw[name] = nc.dram_tensor(name, shape, f32, kind="ExternalInput")

    with TileContext(nc) as tc:
        with (
            tc.tile_pool(name="wpool", bufs=1) as wp,
            tc.tile_pool(name="work", bufs=1) as wk,
            tc.tile_pool(name="micro", bufs=1) as mp,
            tc.tile_pool(name="ps1", bufs=3, space="PSUM") as ps1,   # q/dt
            tc.tile_pool(name="ps2", bufs=2, space="PSUM") as ps2,   # mm2
            tc.tile_pool(name="ps3", bufs=2, space="PSUM") as ps3,   # st/xdc
            tc.tile_pool(name="ps4", bufs=1, space="PSUM") as ps4,   # xd/bcm
        ):
            # ---- load weights once ----
            wt = {}
            for name, shape in [("wbigT_lo", (128, 576)), ("wbigT_hi", (64, 576)),
                                ("xpmT_lo", (128, 96)), ("xpmT_hi", (64, 96)),
                                ("xpcT_lo", (128, 96)), ("xpcT_hi", (64, 96)),
                                ("wdtT", (48, 768)), ("combT", (96, 128)),
                                ("selsA", (128, 512)), ("selsB", (128, 128)),
                                ("wspT_lo", (128, 192)), ("wspT_hi", (64, 192)),
                                ("woT_lo", (128, 192)), ("woT_hi", (64, 192)),
                                ("ones_lo", (128, 1)), ("ones_hi", (64, 1)),
                                ("pcol", (128, NPCOL))]:
                wt[name] = wp.tile(list(shape), f32, tag=name, name=name)
            for nm, src in [("wbigT_lo", dw["wbigT"][0:128, :]),
                            ("wbigT_hi", dw["wbigT"][128:192, :]),
                            ("xpmT_lo", dw["xpmT"][0:128, :]),
                            ("xpmT_hi", dw["xpmT"][128:192, :]),
                            ("xpcT_lo", dw["xpcT"][0:128, :]),
                            ("xpcT_hi", dw["xpcT"][128:192, :]),
                            ("wdtT", dw["wdtT"][:]),
                            ("combT", dw["combT"][:]),
                            ("selsA", dw["selsA"][:]),
                            ("selsB", dw["selsB"][:]),
                            ("wspT_lo", dw["wspT"][0:128, :]),
                            ("wspT_hi", dw["wspT"][128:192, :]),
                            ("woT_lo", dw["woT"][0:128, :]),
                            ("woT_hi", dw["woT"][128:192, :]),
                            ("ones_lo", dw["onesc"][0:128, :]),
                            ("ones_hi", dw["onesc"][128:192, :]),
                            ("pcol", dw["pcol"][:])]:
                nc.sync.dma_start(wt[nm][:], src)

            PC = {}
            idx = 0
            for m in range(6):
                PC[f"negb{m}"] = idx; idx += 1
                PC[f"b{m}"] = idx; idx += 1
            for j in range(6):
                PC[f"dtb{j}"] = idx; idx += 1
            for nm in ["DsA", "DsB", "obpA", "obpB", "ceps", "hlnc"]:
                PC[nm] = idx; idx += 1

            def pc(name, rows=128):
                return wt["pcol"][0:rows, PC[name]:PC[name] + 1]

            def ln_stats(xl, xh, sql, sqh, tag):
                """(rstd bcast [128,NT], mu*rstd bcast [128,NT])"""
                st_s = ps3.tile([1, NT], f32, tag="st", name=f"st_s{tag}")
                st_q = ps3.tile([1, NT], f32, tag="st", name=f"st_q{tag}")
                nc.tensor.matmul(st_s[:], wt["ones_lo"][:], xl[:], start=True, stop=False)
                nc.tensor.matmul(st_s[:], wt["ones_hi"][:], xh[:], start=False, stop=True)
                nc.tensor.matmul(st_q[:], wt["ones_lo"][:], sql[:], start=True, stop=False)
                nc.tensor.matmul(st_q[:], wt["ones_hi"][:], sqh[:], start=False, stop=True)
                mu = mp.tile([1, NT], f32, tag=f"mu{tag}", name=f"mu{tag}")
                nc.vector.tensor_scalar_mul(mu[:], st_s[:], 1.0 / C)
                p1 = mp.tile([1, NT], f32, tag=f"p1{tag}", name=f"p1{tag}")
                nc.vector.tensor_tensor(p1[:], st_s[:], mu[:], OP.mult)
                raw = mp.tile([1, NT], f32, tag=f"raw{tag}", name=f"raw{tag}")
                nc.vector.tensor_tensor(raw[:], st_q[:], p1[:], OP.subtract)
                lv = mp.tile([1, NT], f32, tag=f"lv{tag}", name=f"lv{tag}")
                nc.scalar.activation(lv[:], raw[:], AF.Ln, bias=pc("ceps", 1))
                rstd = mp.tile([1, NT], f32, tag=f"rstd{tag}", name=f"rstd{tag}")
                nc.scalar.activation(rstd[:], lv[:], AF.Exp, bias=pc("hlnc", 1),
                                     scale=-0.5)
                mur = mp.tile([1, NT], f32, tag=f"mur{tag}", name=f"mur{tag}")
                nc.vector.tensor_tensor(mur[:], mu[:], rstd[:], OP.mult)
                rb = wk.tile([128, NT], f32, tag=f"rb{tag}", name=f"rb{tag}")
                mb = wk.tile([128, NT], f32, tag=f"mb{tag}", name=f"mb{tag}")
                nc.gpsimd.partition_broadcast(rb[:], rstd[:])
                nc.gpsimd.partition_broadcast(mb[:], mur[:])
                return rb, mb

            def silu_from_psum(ps, rows, m, out):
                """out = silu(ps + b_m); all on partitions [0, rows)."""
                e = wk.tile([128, NT], f32, tag="sil_e", name="sil_e", bufs=2)
                nc.scalar.activation(e[0:rows, :], ps[0:rows, :], AF.Exp,
                                     bias=pc(f"negb{m}", rows), scale=-1.0)
                wv = wk.tile([128, NT], f32, tag="sil_w", name="sil_w", bufs=2)
                nc.gpsimd.tensor_scalar_add(wv[0:rows, :], e[0:rows, :], 1.0)
                r = wk.tile([128, NT], f32, tag="sil_r", name="sil_r", bufs=2)
                nc.vector.reciprocal_approx_fast(r[0:rows, :], wv[0:rows, :])
                nc.vector.scalar_tensor_tensor(
                    out[0:rows, :], ps[0:rows, :], pc(f"b{m}", rows),
                    r[0:rows, :], OP.add, OP.mult)

            # ================= main tile loop =================
            for t in range(NTILES):
                ts = slice(t * NT, (t + 1) * NT)
                xl = wk.tile([128, NT], f32, tag="xl", name="xl", bufs=2)
                xh = wk.tile([64, NT], f32, tag="xh", name="xh", bufs=2)
                nc.sync.dma_start(xl[:], xin[0:128, ts])
                nc.sync.dma_start(xh[:], xin[128:192, ts])

                sql = wk.tile([128, NT], f32, tag="sql", name="sql")
                sqh = wk.tile([64, NT], f32, tag="sqh", name="sqh")
                nc.gpsimd.tensor_mul(sql[:], xl[:], xl[:])
                nc.gpsimd.tensor_mul(sqh[:], xh[:], xh[:])

                rb, mb = ln_stats(xl, xh, sql, sqh, "1")

                # x_hat = x*rstd - mu*rstd
                xhl = wk.tile([128, NT], f32, tag="xhl", name="xhl")
                xhh = wk.tile([64, NT], f32, tag="xhh", name="xhh")
                nc.vector.tensor_tensor(xhl[:], xl[:], rb[:], OP.mult)
                nc.vector.tensor_tensor(xhl[:], xhl[:], mb[0:128, :], OP.subtract)
                nc.vector.tensor_tensor(xhh[:], xh[:], rb[0:64, :], OP.mult)
                nc.vector.tensor_tensor(xhh[:], xhh[:], mb[0:64, :], OP.subtract)

                # fused big matmul -> 6 chunks aligned with consumers
                q = []
                for m, (r0, r1) in enumerate(QCH):
                    rows = r1 - r0
                    ps = ps1.tile([rows, NT], f32, tag="q", name=f"q{m}")
                    nc.tensor.matmul(ps[:], wt["wbigT_lo"][:, r0:r1], xhl[:],
                                     start=True, stop=False)
                    nc.tensor.matmul(ps[:], wt["wbigT_hi"][:, r0:r1], xhh[:],
                                     start=False, stop=True)
                    q.append(ps)

                xssA = wk.tile([128, NT], f32, tag="xssA", name="xssA")
                xssB = wk.tile([64, NT], f32, tag="xssB", name="xssB")
                zA = wk.tile([128, NT], f32, tag="zA", name="zA")
                zB = wk.tile([64, NT], f32, tag="zB", name="zB")
                sx2a = wk.tile([128, NT], f32, tag="sx2a", name="sx2a")
                sx2b = wk.tile([64, NT], f32, tag="sx2b", name="sx2b")
                for m, out in enumerate([xssA, xssB, zA, zB, sx2a, sx2b]):
                    silu_from_psum(q[m], QCH[m][1] - QCH[m][0], m, out)

                # xd_main = [dtr;pad;Bv] @ xss ; xd_cv = [0;Cv] @ xss
                xdp = ps4.tile([96, NT], f32, tag="xdb", name="xdp")
                nc.tensor.matmul(xdp[:], wt["xpmT_lo"][:], xssA[:], start=True, stop=False)
                nc.tensor.matmul(xdp[:], wt["xpmT_hi"][:], xssB[:], start=False, stop=True)
                xdc = ps3.tile([96, NT], f32, tag="st", name="xdc")
                nc.tensor.matmul(xdc[:], wt["xpcT_lo"][:], xssA[:], start=True, stop=False)
                nc.tensor.matmul(xdc[:], wt["xpcT_hi"][:], xssB[:], start=False, stop=True)
                xds = wk.tile([96, NT], f32, tag="xds", name="xds")
                nc.vector.tensor_copy(xds[:], xdp[:])

                # Bv*Cv at partitions 64:96 (psum operand: no SB base rule)
                bcpt = wk.tile([96, NT], f32, tag="bcpt", name="bcpt")
                nc.vector.tensor_tensor(bcpt[64:96, :], xds[64:96, :],
                                        xdc[64:96, :], OP.mult)
                bcm_ps = ps4.tile([128, NT], f32, tag="xdb", name="bcm_ps")
                nc.tensor.matmul(bcm_ps[:], wt["combT"][64:96, :], bcpt[64:96, :])
                bcm = wk.tile([128, NT], f32, tag="bcm", name="bcm")
                nc.vector.tensor_copy(bcm[:], bcm_ps[:])

                # dtpre chunks -> softplus -> *bc
                pj = []
                for j in range(6):
                    dtp = ps1.tile([128, NT], f32, tag="q", name=f"dt{j}")
                    nc.tensor.matmul(dtp[:], wt["wdtT"][:, 128 * j:128 * (j + 1)],
                                     xds[0:48, :])
                    et = wk.tile([128, NT], f32, tag="et", name="et", bufs=2)
                    nc.scalar.activation(et[:], dtp[:], AF.Exp, bias=pc(f"dtb{j}"))
                    sp = wk.tile([128, NT], f32, tag="sp", name="sp", bufs=2)
                    nc.scalar.activation(sp[:], et[:], AF.Ln, bias=1.0)
                    pp = wk.tile([128, NT], f32, tag=f"pj{j % 3}", name=f"pj{j}",
                                 bufs=2)
                    if j % 2 == 0:
                        nc.vector.tensor_tensor(pp[:], sp[:], bcm[:], OP.mult)
                    else:
                        nc.gpsimd.tensor_mul(pp[:], sp[:], bcm[:])
                    pj.append(pp)

                gA = ps2.tile([128, NT], f32, tag="mm2", name="gA")
                for j in range(4):
                    nc.tensor.matmul(gA[:], wt["selsA"][:, 128 * j:128 * (j + 1)],
                                     pj[j][:], start=(j == 0), stop=(j == 3))
                gB = ps2.tile([64, NT], f32, tag="mm2", name="gB")
                for j in range(2):
                    nc.tensor.matmul(gB[:], wt["selsB"][:, 64 * j:64 * (j + 1)],
                                     pj[4 + j][:], start=(j == 0), stop=(j == 1))

                # y = (gain + Dsum) * xss
                yA = wk.tile([128, NT], f32, tag="yA", name="yA")
                yB = wk.tile([64, NT], f32, tag="yB", name="yB")
                nc.vector.scalar_tensor_tensor(yA[:], gA[:], pc("DsA"), xssA[:],
                                               OP.add, OP.mult)
                nc.vector.scalar_tensor_tensor(yB[:], gB[:], pc("DsB", 64), xssB[:],
                                               OP.add, OP.mult)

                ysqA = wk.tile([128, NT], f32, tag="ysqA", name="ysqA")
                ysqB = wk.tile([64, NT], f32, tag="ysqB", name="ysqB")
                nc.gpsimd.tensor_mul(ysqA[:], yA[:], yA[:])
                nc.gpsimd.tensor_mul(ysqB[:], yB[:], yB[:])
                rb2, mb2 = ln_stats(yA, yB, ysqA, ysqB, "2")

                # v = (ln2(y) + obp) * z
                vA = wk.tile([128, NT], f32, tag="vA", name="vA")
                vB = wk.tile([64, NT], f32, tag="vB", name="vB")
                nc.vector.tensor_tensor(vA[:], yA[:], rb2[:], OP.mult)
                nc.vector.tensor_tensor(vA[:], vA[:], mb2[0:128, :], OP.subtract)
                nc.vector.scalar_tensor_tensor(vA[:], vA[:], pc("obpA"), zA[:],
                                               OP.add, OP.mult)
                nc.gpsimd.tensor_mul(vB[:], yB[:], rb2[0:64, :])
                nc.gpsimd.tensor_sub(vB[:], vB[:], mb2[0:64, :])
                nc.vector.scalar_tensor_tensor(vB[:], vB[:], pc("obpB", 64), zB[:],
                                               OP.add, OP.mult)

                # s = Wsp @ v
                sA = ps2.tile([128, NT], f32, tag="mm2", name="sA")
                sB = ps2.tile([64, NT], f32, tag="mm2", name="sB")
                nc.tensor.matmul(sA[:], wt["wspT_lo"][:, 0:128], vA[:], start=True, stop=False)
                nc.tensor.matmul(sA[:], wt["wspT_hi"][:, 0:128], vB[:], start=False, stop=True)
                nc.tensor.matmul(sB[:], wt["wspT_lo"][:, 128:192], vA[:], start=True, stop=False)
                nc.tensor.matmul(sB[:], wt["wspT_hi"][:, 128:192], vB[:], start=False, stop=True)

                gsA = wk.tile([128, NT], f32, tag="gsA", name="gsA")
                gsB = wk.tile([64, NT], f32, tag="gsB", name="gsB")
                nc.vector.tensor_tensor(gsA[:], sA[:], sx2a[:], OP.mult)
                nc.vector.tensor_tensor(gsB[:], sB[:], sx2b[:], OP.mult)

                oA = ps2.tile([128, NT], f32, tag="mm2", name="oA")
                oB = ps2.tile([64, NT], f32, tag="mm2", name="oB")
                nc.tensor.matmul(oA[:], wt["woT_lo"][:, 0:128], gsA[:], start=True, stop=False)
                nc.tensor.matmul(oA[:], wt["woT_hi"][:, 0:128], gsB[:], start=False, stop=True)
                nc.tensor.matmul(oB[:], wt["woT_lo"][:, 128:192], gsA[:], start=True, stop=False)
                nc.tensor.matmul(oB[:], wt["woT_hi"][:, 128:192], gsB[:], start=False, stop=True)

                outA = wk.tile([128, NT], f32, tag="outA", name="outA", bufs=2)
                outB = wk.tile([64, NT], f32, tag="outB", name="outB", bufs=2)
                nc.vector.tensor_tensor(outA[:], oA[:], xl[:], OP.add)
                nc.vector.tensor_tensor(outB[:], oB[:], xh[:], OP.add)
                nc.sync.dma_start(yout[0:128, ts], outA[:])
                nc.sync.dma_start(yout[128:192, ts], outB[:])

    nc.compile()
    return nc


def _get_program():
    if "nc" not in _CACHE:
        _CACHE["nc"] = _build_program()
    return _CACHE["nc"]


def _shard(x):
    """x (B,C,H,W) -> list of 8 (C, NPIX) channel-major slices"""
    outs = []
    for i in range(NCORES):
        b, h0 = i // 2, (i % 2) * (H // 2)
        outs.append(np.ascontiguousarray(
            x[b, :, h0:h0 + H // 2, :].reshape(C, NPIX)))
    return outs


def _unshard(parts, dtype):
    out = np.empty((B, C, H, W), dtype)
    for i in range(NCORES):
        b, h0 = i // 2, (i % 2) * (H // 2)
        out[b, :, h0:h0 + H // 2, :] = parts[i].reshape(C, H // 2, W)
    return out


def run(inputs, trace=False):
    from concourse.bass_utils import run_bass_kernel_spmd
    nc = _get_program()
    wts, _ = _fold_weights(**{k: np.asarray(v) for k, v in inputs.items()
                              if k != "x"})
    shards = _shard(np.asarray(inputs["x"], np.float32))
    in_maps = [dict(wts, xin=s) for s in shards]
    res = run_bass_kernel_spmd(nc, in_maps, core_ids=list(range(NCORES)),
                               trace=trace)
    out = _unshard([r["yout"] for r in res.results], np.float32)
    return out, res


def kernel(**inputs):
    out, _ = run(inputs, trace=False)
    return out


if __name__ == "__main__":
    print("building program...")
    nc = _get_program()
    print("program built")


# revision 15
# speedup vs baseline: 1.6975x; 1.6975x over previous
"""Trainium2 Bass kernel for nn_ChannelMambaBlock.

Math (per pixel, channel vector x of size C=192):
  xn  = LN(x)*nw + nb
  p   = w_in @ xn              ; x1, x2 = p[:C], p[C:]
  u   = ssm_in @ x1            ; xss, z = silu(u[:C]), silu(u[C:])
  xd  = x_proj @ xss (K dirs)  ; dtr, Bv, Cv
  dt  = softplus(dt_w @ dtr + dt_b)
  bc_k = sum_s Bv*Cv ; gain = sum_k bc_k*dt_k + sum_k D_k
  y   = LN2(xss*gain)*ow + ob ; y *= z
  s   = ssm_out @ y ; o = w_out @ (s * silu(x2)) ; out = x + o

Kernel strategy: channel-major [C, pixels] layout, pure data parallel over
8 cores (each owns B*H*W/8 = 8192 pixels, tiles of 512 pixels).
  - LN affine + w_in + ssm_in folded on host into one [576, C] matmul
    producing [xss_pre, z_pre, x2_pre]; per-channel biases ride as
    per-partition ACT bias operands.
  - M-chunks of that matmul coincide with consumer groups (128+64 per
    group) so every elementwise op is partition-aligned (DVE cannot move
    data across partitions).
  - LN stats via PE ones-matmul; rsqrt via exp(-0.5*ln(var+eps)) on ACT.
  - silu via exp(-v) on ACT + reciprocal_approx_fast on DVE (single ACT
    table set: natural_log_exp_and_others -> no table switching).
  - softplus via ACT: Ln(Exp(dtpre + dt_b) + 1).
  - dt block-diag matmul in c-major row order (row = c*4+k); gain k-sum
    on PE with 0/1 selector matmuls; bc broadcast to the c*4+k pattern
    with one selector matmul.
  - Per-pixel scalars (rstd, mu*rstd) broadcast across partitions with
    GPSIMD partition_broadcast.
"""

import numpy as np

C = 192
K = 4
DT = 12
DS = 8
B, H, W = 4, 128, 128
EPS = 1e-5
NCORES = 8
NPIX = B * H * W // NCORES      # 8192 per core
NT = 512                        # pixels per tile
NTILES = NPIX // NT             # 16

# M-chunks of the big fused matmul [u(384); x2(192)]:
#   q0 xss[0:128], q1 xss[128:192], q2 z[0:128], q3 z[128:192],
#   q4 x2[0:128], q5 x2[128:192]
QCH = [(0, 128), (128, 192), (192, 320), (320, 384), (384, 512), (512, 576)]

NPCOL = 24

_CACHE = {}


def _fold_weights(norm_w, norm_b, w_in, ssm_in_w, x_proj_w, dt_w, dt_b,
                  A_logs, Ds, out_norm_w, out_norm_b, ssm_out_w, w_out):
    f8 = np.float64
    nw, nb = norm_w.astype(f8), norm_b.astype(f8)
    w_in = w_in.astype(f8)
    ssm = ssm_in_w.astype(f8)
    W1 = w_in * nw[None, :]
    b1 = w_in @ nb
    W_u = ssm @ W1[:C]                      # (2C, C)
    b_u = ssm @ b1[:C]                      # (2C,)
    W1b = W1[C:]                            # (C, C)
    b_x2 = b1[C:]
    W_big = np.concatenate([W_u, W1b], 0)   # (576, C)
    b_big = np.concatenate([b_u, b_x2], 0)  # (576,)
    # x_proj main rows: [dtr (48, k-major) ; pad (16) ; Bv (32)] = 96
    Xpm = np.concatenate([
        x_proj_w[:, :DT].reshape(K * DT, C),
        np.zeros((16, C)),
        x_proj_w[:, DT:DT + DS].reshape(K * DS, C)], 0).astype(f8)   # (96, C)
    # Cv matmul: rows [pad(64) ; Cv(32)] so Cv lands on partitions 64:96
    Xpc = np.concatenate([
        np.zeros((64, C)),
        x_proj_w[:, DT + DS:].reshape(K * DS, C)], 0).astype(f8)     # (96, C)
    # dt block: out row = c*4 + k, in col = k*12 + r
    Wdt = np.zeros((C * K, K * DT))
    for k in range(K):
        Wdt[np.arange(C) * K + k, k * DT:(k + 1) * DT] = dt_w[k].astype(f8)
    dtb_s = dt_b.astype(f8).T.reshape(C * K)  # row c*4+k
    Dsum = Ds.astype(f8).reshape(K, C).sum(0)  # (C,)
    ow, ob = out_norm_w.astype(f8), out_norm_b.astype(f8)
    Wsp = ssm_out_w.astype(f8) * ow[None, :]
    obp = ob / ow
    Wo = w_out.astype(f8)

    # bc pattern broadcast: bcm[m] = bc[m%4]; lhsT rows at partitions 64:96
    # (matching where Bv*Cv sits).
    combT = np.zeros((96, 128))
    for k in range(K):
        rows = 64 + k * DS + np.arange(DS)
        combT[np.ix_(rows, np.arange(128)[np.arange(128) % 4 == k])] = 1.0
    # gain selectors: chunk j of dt rows [128j,128j+128) covers c in
    # [32j, 32j+32): gain_sel_j[p, 32j + p//4] = 1
    selsA = []
    for j in range(4):
        S = np.zeros((128, 128))
        S[np.arange(128), 32 * j + np.arange(128) // 4] = 1.0
        selsA.append(S)
    selsB = []
    for j in range(2):
        S = np.zeros((128, 64))
        S[np.arange(128), 32 * j + np.arange(128) // 4] = 1.0
        selsB.append(S)

    def f32(a):
        return np.ascontiguousarray(np.asarray(a, np.float32))

    import ml_dtypes

    def bf(a):
        return np.ascontiguousarray(np.asarray(a).astype(ml_dtypes.bfloat16))

    wts = {
        "wbigT": bf(W_big.T),               # (192, 576)
        "xpmT": bf(Xpm.T),                  # (192, 96)
        "xpcT": bf(Xpc.T),                  # (192, 96)
        "wdtT": bf(Wdt.T),                  # (48, 768)
        "combT": bf(combT),                 # (96, 128)
        "selsA": bf(np.concatenate(selsA, 1)),   # (128, 512)
        "selsB": bf(np.concatenate(selsB, 1)),   # (128, 128)
        "wspT": bf(Wsp.T),                  # (192, 192)
        "woT": bf(Wo.T),                    # (192, 192)
        "onesc": bf(np.ones((C, 1))),       # (192, 1) stats lhsT
    }
    cols = []

    def col(v):
        v = np.asarray(v, np.float64).reshape(-1)
        c = np.zeros(128)
        c[:v.size] = v
        cols.append(c)
        return len(cols) - 1

    ci = {}
    for m, (r0, r1) in enumerate(QCH):
        ci[f"negb{m}"] = col(-b_big[r0:r1])
        ci[f"b{m}"] = col(b_big[r0:r1])
    for j in range(6):
        ci[f"dtb{j}"] = col(dtb_s[128 * j:128 * (j + 1)])
    ci["DsA"] = col(Dsum[:128]); ci["DsB"] = col(Dsum[128:])
    ci["obpA"] = col(obp[:128]); ci["obpB"] = col(obp[128:])
    ci["ceps"] = col(np.full(128, C * EPS))
    ci["hlnc"] = col(np.full(128, 0.5 * np.log(C)))
    assert len(cols) == NPCOL, len(cols)
    wts["pcol"] = f32(np.stack(cols, 1))     # (128, NPCOL)
    return wts, ci


def _build_program():
    import concourse.mybir as mybir
    from concourse.bacc import Bacc
    from concourse.tile import TileContext
    from concourse.alu_op_type import AluOpType

    AF = mybir.ActivationFunctionType
    OP = AluOpType
    f32 = mybir.dt.float32
    bf = mybir.dt.bfloat16
    nc = Bacc()

    xin = nc.dram_tensor("xin", (C, NPIX), f32, kind="ExternalInput")
    yout = nc.dram_tensor("yout", (C, NPIX), f32, kind="ExternalOutput")
    dw = {}
    for name, shape in [("wbigT", (C, 576)), ("xpmT", (C, 96)),
                        ("xpcT", (C, 96)),
                        ("wdtT", (48, 768)), ("combT", (96, 128)),
                        ("selsA", (128, 512)), ("selsB", (128, 128)),
                        ("wspT", (C, 192)), ("woT", (C, 192)),
                        ("onesc", (C, 1))]:
        dw[name] = nc.dram_tensor(name, shape, bf, kind="ExternalInput")
    dw["pcol"] = nc.dram_tensor("pcol", (128, NPCOL), f32, kind="ExternalInput")

    with TileContext(nc) as tc:
        with (
            tc.tile_pool(name="wpool", bufs=1) as wp,
            tc.tile_pool(name="work", bufs=1) as wk,
            tc.tile_pool(name="micro", bufs=1) as mp,
            tc.tile_pool(name="ps1", bufs=3, space="PSUM") as ps1,   # q/dt
            tc.tile_pool(name="ps2", bufs=2, space="PSUM") as ps2,   # mm2
            tc.tile_pool(name="ps3", bufs=2, space="PSUM") as ps3,   # st/xdc
            tc.tile_pool(name="ps4", bufs=1, space="PSUM") as ps4,   # xd/bcm
        ):
            # ---- load weights once ----
            wt = {}
            for name, shape in [("wbigT_lo", (128, 576)), ("wbigT_hi", (64, 576)),
                                ("xpmT_lo", (128, 96)), ("xpmT_hi", (64, 96)),
                                ("xpcT_lo", (128, 96)), ("xpcT_hi", (64, 96)),
                                ("wdtT", (48, 768)), ("combT", (96, 128)),
                                ("selsA", (128, 512)), ("selsB", (128, 128)),
                                ("wspT_lo", (128, 192)), ("wspT_hi", (64, 192)),
                                ("woT_lo", (128, 192)), ("woT_hi", (64, 192)),
                                ("ones_lo", (128, 1)), ("ones_hi", (64, 1))]:
                wt[name] = wp.tile(list(shape), bf, tag=name, name=name)
            wt["pcol"] = wp.tile([128, NPCOL], f32, tag="pcol", name="pcol")
            for nm, src in [("wbigT_lo", dw["wbigT"][0:128, :]),
                            ("wbigT_hi", dw["wbigT"][128:192, :]),
                            ("xpmT_lo", dw["xpmT"][0:128, :]),
                            ("xpmT_hi", dw["xpmT"][128:192, :]),
                            ("xpcT_lo", dw["xpcT"][0:128, :]),
                            ("xpcT_hi", dw["xpcT"][128:192, :]),
                            ("wdtT", dw["wdtT"][:]),
                            ("combT", dw["combT"][:]),
                            ("selsA", dw["selsA"][:]),
                            ("selsB", dw["selsB"][:]),
                            ("wspT_lo", dw["wspT"][0:128, :]),
                            ("wspT_hi", dw["wspT"][128:192, :]),
                            ("woT_lo", dw["woT"][0:128, :]),
                            ("woT_hi", dw["woT"][128:192, :]),
                            ("ones_lo", dw["onesc"][0:128, :]),
                            ("ones_hi", dw["onesc"][128:192, :]),
                            ("pcol", dw["pcol"][:])]:
                nc.sync.dma_start(wt[nm][:], src)

            PC = {}
            idx = 0
            for m in range(6):
                PC[f"negb{m}"] = idx; idx += 1
                PC[f"b{m}"] = idx; idx += 1
            for j in range(6):
                PC[f"dtb{j}"] = idx; idx += 1
            for nm in ["DsA", "DsB", "obpA", "obpB", "ceps", "hlnc"]:
                PC[nm] = idx; idx += 1

            def pc(name, rows=128):
                return wt["pcol"][0:rows, PC[name]:PC[name] + 1]

            def ln_stats(xl, xh, sql, sqh, tag):
                """(rstd bcast [128,NT], mu*rstd bcast [128,NT])"""
                st_s = ps3.tile([1, NT], f32, tag="st", name=f"st_s{tag}")
                st_q = ps3.tile([1, NT], f32, tag="st", name=f"st_q{tag}")
                nc.tensor.matmul(st_s[:], wt["ones_lo"][:], xl[:], start=True, stop=False)
                nc.tensor.matmul(st_s[:], wt["ones_hi"][:], xh[:], start=False, stop=True)
                nc.tensor.matmul(st_q[:], wt["ones_lo"][:], sql[:], start=True, stop=False)
                nc.tensor.matmul(st_q[:], wt["ones_hi"][:], sqh[:], start=False, stop=True)
                mu = mp.tile([1, NT], f32, tag=f"mu{tag}", name=f"mu{tag}")
                nc.vector.tensor_scalar_mul(mu[:], st_s[:], 1.0 / C)
                p1 = mp.tile([1, NT], f32, tag=f"p1{tag}", name=f"p1{tag}")
                nc.vector.tensor_tensor(p1[:], st_s[:], mu[:], OP.mult)
                raw = mp.tile([1, NT], f32, tag=f"raw{tag}", name=f"raw{tag}")
                nc.vector.tensor_tensor(raw[:], st_q[:], p1[:], OP.subtract)
                lv = mp.tile([1, NT], f32, tag=f"lv{tag}", name=f"lv{tag}")
                nc.scalar.activation(lv[:], raw[:], AF.Ln, bias=pc("ceps", 1))
                rstd = mp.tile([1, NT], f32, tag=f"rstd{tag}", name=f"rstd{tag}")
                nc.scalar.activation(rstd[:], lv[:], AF.Exp, bias=pc("hlnc", 1),
                                     scale=-0.5)
                mur = mp.tile([1, NT], f32, tag=f"mur{tag}", name=f"mur{tag}")
                nc.vector.tensor_tensor(mur[:], mu[:], rstd[:], OP.mult)
                rb = wk.tile([128, NT], f32, tag=f"rb{tag}", name=f"rb{tag}")
                mb = wk.tile([128, NT], f32, tag=f"mb{tag}", name=f"mb{tag}")
                nc.gpsimd.partition_broadcast(rb[:], rstd[:])
                nc.gpsimd.partition_broadcast(mb[:], mur[:])
                return rb, mb

            def silu_from_psum(ps, rows, m, out):
                """out = silu(ps + b_m); all on partitions [0, rows)."""
                e = wk.tile([128, NT], f32, tag="sil_e", name="sil_e", bufs=2)
                nc.scalar.activation(e[0:rows, :], ps[0:rows, :], AF.Exp,
                                     bias=pc(f"negb{m}", rows), scale=-1.0)
                wv = wk.tile([128, NT], f32, tag="sil_w", name="sil_w", bufs=2)
                nc.vector.tensor_scalar_add(wv[0:rows, :], e[0:rows, :], 1.0)
                r = wk.tile([128, NT], f32, tag="sil_r", name="sil_r", bufs=2)
                nc.vector.reciprocal_approx_fast(r[0:rows, :], wv[0:rows, :])
                nc.vector.scalar_tensor_tensor(
                    out[0:rows, :], ps[0:rows, :], pc(f"b{m}", rows),
                    r[0:rows, :], OP.add, OP.mult)

            # ================= main tile loop =================
            for t in range(NTILES):
                ts = slice(t * NT, (t + 1) * NT)
                xl = wk.tile([128, NT], f32, tag="xl", name="xl", bufs=2)
                xh = wk.tile([64, NT], f32, tag="xh", name="xh", bufs=2)
                nc.sync.dma_start(xl[:], xin[0:128, ts])
                nc.sync.dma_start(xh[:], xin[128:192, ts])

                xbl = wk.tile([128, NT], bf, tag="xbl", name="xbl", bufs=2)
                xbh = wk.tile([64, NT], bf, tag="xbh", name="xbh", bufs=2)
                nc.vector.tensor_copy(xbl[:], xl[:])
                nc.vector.tensor_copy(xbh[:], xh[:])
                sql = wk.tile([128, NT], bf, tag="sql", name="sql", bufs=2)
                sqh = wk.tile([64, NT], bf, tag="sqh", name="sqh", bufs=2)
                nc.gpsimd.tensor_mul(sql[:], xbl[:], xbl[:])
                nc.gpsimd.tensor_mul(sqh[:], xbh[:], xbh[:])

                rb, mb = ln_stats(xbl, xbh, sql, sqh, "1")

                # x_hat = x*rstd - mu*rstd  (bf16 out on the final op)
                xt = wk.tile([128, NT], f32, tag="xt", name="xt")
                xhl = wk.tile([128, NT], bf, tag="xhl", name="xhl", bufs=2)
                xhh = wk.tile([64, NT], bf, tag="xhh", name="xhh", bufs=2)
                nc.vector.tensor_tensor(xt[:], xl[:], rb[:], OP.mult)
                nc.vector.tensor_tensor(xhl[:], xt[:], mb[0:128, :], OP.subtract)
                nc.vector.tensor_tensor(xt[0:64, :], xh[:], rb[0:64, :], OP.mult)
                nc.vector.tensor_tensor(xhh[:], xt[0:64, :], mb[0:64, :], OP.subtract)

                # fused big matmul -> 6 chunks aligned with consumers
                q = []
                for m, (r0, r1) in enumerate(QCH):
                    rows = r1 - r0
                    ps = ps1.tile([rows, NT], f32, tag="q", name=f"q{m}")
                    nc.tensor.matmul(ps[:], wt["wbigT_lo"][:, r0:r1], xhl[:],
                                     start=True, stop=False)
                    nc.tensor.matmul(ps[:], wt["wbigT_hi"][:, r0:r1], xhh[:],
                                     start=False, stop=True)
                    q.append(ps)

                xssA = wk.tile([128, NT], bf, tag="xssA", name="xssA", bufs=2)
                xssB = wk.tile([64, NT], bf, tag="xssB", name="xssB", bufs=2)
                zA = wk.tile([128, NT], bf, tag="zA", name="zA", bufs=2)
                zB = wk.tile([64, NT], bf, tag="zB", name="zB", bufs=2)
                sx2a = wk.tile([128, NT], bf, tag="sx2a", name="sx2a", bufs=2)
                sx2b = wk.tile([64, NT], bf, tag="sx2b", name="sx2b", bufs=2)
                for m, out in enumerate([xssA, xssB, zA, zB, sx2a, sx2b]):
                    silu_from_psum(q[m], QCH[m][1] - QCH[m][0], m, out)

                # xd_main = [dtr;pad;Bv] @ xss ; xd_cv = [0;Cv] @ xss
                xdp = ps4.tile([96, NT], f32, tag="xdb", name="xdp")
                nc.tensor.matmul(xdp[:], wt["xpmT_lo"][:], xssA[:], start=True, stop=False)
                nc.tensor.matmul(xdp[:], wt["xpmT_hi"][:], xssB[:], start=False, stop=True)
                xdc = ps3.tile([96, NT], f32, tag="st", name="xdc")
                nc.tensor.matmul(xdc[:], wt["xpcT_lo"][:], xssA[:], start=True, stop=False)
                nc.tensor.matmul(xdc[:], wt["xpcT_hi"][:], xssB[:], start=False, stop=True)
                xds = wk.tile([96, NT], bf, tag="xds", name="xds", bufs=2)
                nc.vector.tensor_copy(xds[:], xdp[:])

                # Bv*Cv at partitions 64:96 (psum operand: no SB base rule)
                bcpt = wk.tile([96, NT], bf, tag="bcpt", name="bcpt", bufs=2)
                nc.vector.tensor_tensor(bcpt[64:96, :], xds[64:96, :],
                                        xdc[64:96, :], OP.mult)
                bcm_ps = ps4.tile([128, NT], f32, tag="xdb", name="bcm_ps")
                nc.tensor.matmul(bcm_ps[:], wt["combT"][64:96, :], bcpt[64:96, :])
                bcm = wk.tile([128, NT], bf, tag="bcm", name="bcm", bufs=2)
                nc.vector.tensor_copy(bcm[:], bcm_ps[:])

                # dtpre chunks -> softplus -> *bc
                pj = []
                for j in range(6):
                    dtp = ps1.tile([128, NT], f32, tag="q", name=f"dt{j}")
                    nc.tensor.matmul(dtp[:], wt["wdtT"][:, 128 * j:128 * (j + 1)],
                                     xds[0:48, :])
                    et = wk.tile([128, NT], f32, tag="et", name="et", bufs=2)
                    nc.scalar.activation(et[:], dtp[:], AF.Exp, bias=pc(f"dtb{j}"))
                    sp = wk.tile([128, NT], bf, tag="sp", name="sp", bufs=2)
                    nc.scalar.activation(sp[:], et[:], AF.Ln, bias=1.0)
                    pp = wk.tile([128, NT], bf, tag=f"pj{j % 3}", name=f"pj{j}",
                                 bufs=2)
                    if j % 2 == 0:
                        nc.vector.tensor_tensor(pp[:], sp[:], bcm[:], OP.mult)
                    else:
                        nc.gpsimd.tensor_mul(pp[:], sp[:], bcm[:])
                    pj.append(pp)

                gA = ps2.tile([128, NT], f32, tag="mm2", name="gA")
                for j in range(4):
                    nc.tensor.matmul(gA[:], wt["selsA"][:, 128 * j:128 * (j + 1)],
                                     pj[j][:], start=(j == 0), stop=(j == 3))
                gB = ps2.tile([64, NT], f32, tag="mm2", name="gB")
                for j in range(2):
                    nc.tensor.matmul(gB[:], wt["selsB"][:, 64 * j:64 * (j + 1)],
                                     pj[4 + j][:], start=(j == 0), stop=(j == 1))

                # y = (gain + Dsum) * xss
                yA = wk.tile([128, NT], bf, tag="yA", name="yA", bufs=2)
                yB = wk.tile([64, NT], bf, tag="yB", name="yB", bufs=2)
                nc.vector.scalar_tensor_tensor(yA[:], gA[:], pc("DsA"), xssA[:],
                                               OP.add, OP.mult)
                nc.vector.scalar_tensor_tensor(yB[:], gB[:], pc("DsB", 64), xssB[:],
                                               OP.add, OP.mult)

                ysqA = wk.tile([128, NT], bf, tag="ysqA", name="ysqA", bufs=2)
                ysqB = wk.tile([64, NT], bf, tag="ysqB", name="ysqB", bufs=2)
                nc.gpsimd.tensor_mul(ysqA[:], yA[:], yA[:])
                nc.gpsimd.tensor_mul(ysqB[:], yB[:], yB[:])
                rb2, mb2 = ln_stats(yA, yB, ysqA, ysqB, "2")

                # v = (ln2(y) + obp) * z
                vA = wk.tile([128, NT], bf, tag="vA", name="vA", bufs=2)
                vB = wk.tile([64, NT], bf, tag="vB", name="vB", bufs=2)
                nc.vector.tensor_tensor(vA[:], yA[:], rb2[:], OP.mult)
                nc.vector.tensor_tensor(vA[:], vA[:], mb2[0:128, :], OP.subtract)
                nc.vector.scalar_tensor_tensor(vA[:], vA[:], pc("obpA"), zA[:],
                                               OP.add, OP.mult)
                nc.gpsimd.tensor_mul(vB[:], yB[:], rb2[0:64, :])
                nc.gpsimd.tensor_sub(vB[:], vB[:], mb2[0:64, :])
                nc.vector.scalar_tensor_tensor(vB[:], vB[:], pc("obpB", 64), zB[:],
                                               OP.add, OP.mult)

                # s = Wsp @ v
                sA = ps2.tile([128, NT], f32, tag="mm2", name="sA")
                sB = ps2.tile([64, NT], f32, tag="mm2", name="sB")
                nc.tensor.matmul(sA[:], wt["wspT_lo"][:, 0:128], vA[:], start=True, stop=False)
                nc.tensor.matmul(sA[:], wt["wspT_hi"][:, 0:128], vB[:], start=False, stop=True)
                nc.tensor.matmul(sB[:], wt["wspT_lo"][:, 128:192], vA[:], start=True, stop=False)
                nc.tensor.matmul(sB[:], wt["wspT_hi"][:, 128:192], vB[:], start=False, stop=True)

                gsA = wk.tile([128, NT], bf, tag="gsA", name="gsA", bufs=2)
                gsB = wk.tile([64, NT], bf, tag="gsB", name="gsB", bufs=2)
                nc.vector.tensor_tensor(gsA[:], sA[:], sx2a[:], OP.mult)
                nc.vector.tensor_tensor(gsB[:], sB[:], sx2b[:], OP.mult)

                oA = ps2.tile([128, NT], f32, tag="mm2", name="oA")
                oB = ps2.tile([64, NT], f32, tag="mm2", name="oB")
                nc.tensor.matmul(oA[:], wt["woT_lo"][:, 0:128], gsA[:], start=True, stop=False)
                nc.tensor.matmul(oA[:], wt["woT_hi"][:, 0:128], gsB[:], start=False, stop=True)
                nc.tensor.matmul(oB[:], wt["woT_lo"][:, 128:192], gsA[:], start=True, stop=False)
                nc.tensor.matmul(oB[:], wt["woT_hi"][:, 128:192], gsB[:], start=False, stop=True)

                outA = wk.tile([128, NT], f32, tag="outA", name="outA", bufs=2)
                outB = wk.tile([64, NT], f32, tag="outB", name="outB", bufs=2)
                nc.vector.tensor_tensor(outA[:], oA[:], xl[:], OP.add)
                nc.vector.tensor_tensor(outB[:], oB[:], xh[:], OP.add)
                nc.sync.dma_start(yout[0:128, ts], outA[:])
                nc.sync.dma_start(yout[128:192, ts], outB[:])

    nc.compile()
    return nc


def _get_program():
    if "nc" not in _CACHE:
        _CACHE["nc"] = _build_program()
    return _CACHE["nc"]


def _shard(x):
    """x (B,C,H,W) -> list of 8 (C, NPIX) channel-major slices"""
    outs = []
    for i in range(NCORES):
        b, h0 = i // 2, (i % 2) * (H // 2)
        outs.append(np.ascontiguousarray(
            x[b, :, h0:h0 + H // 2, :].reshape(C, NPIX)))
    return outs


def _unshard(parts, dtype):
    out = np.empty((B, C, H, W), dtype)
    for i in range(NCORES):
        b, h0 = i // 2, (i % 2) * (H // 2)
        out[b, :, h0:h0 + H // 2, :] = parts[i].reshape(C, H // 2, W)
    return out


def run(inputs, trace=False):
    from concourse.bass_utils import run_bass_kernel_spmd
    nc = _get_program()
    wts, _ = _fold_weights(**{k: np.asarray(v) for k, v in inputs.items()
                              if k != "x"})
    shards = _shard(np.asarray(inputs["x"], np.float32))
    in_maps = [dict(wts, xin=s) for s in shards]
    res = run_bass_kernel_spmd(nc, in_maps, core_ids=list(range(NCORES)),
                               trace=trace)
    out = _unshard([r["yout"] for r in res.results], np.float32)
    return out, res


def kernel(**inputs):
    out, _ = run(inputs, trace=False)
    return out


if __name__ == "__main__":
    print("building program...")
    nc = _get_program()
    print("program built")
